# revision 47
# baseline (speedup 1.0000x reference)
"""Trainium2 Bass kernel for nn_ContextLabel (GNN label propagation).

Computation: 10 iterations of Y = masked(adj @ Y) on [10000,16], then
straight-through gumbel one-hot, dist = (adj!=0) @ Yh row-normalized,
output mean((dist - pseudo_labels)^2)  (scalar).

The per-step update is affine: Y <- B @ Y + c with B = diag(1-m) adj
and c = m*labels, so two steps fuse into one pass with the squared
operator: Y <- B^2 @ Y + (Bc + c).  The device runs 5 fused passes
(temporal blocking) instead of 10 - identical HBM traffic (one operator
matrix + one mask matrix streamed per core, both fp8), but half the
tensor-engine passes and half the AllGathers.  B^2 is ~810 nnz/row and
is scaled by 64 before fp8 quantization (entries ~1e-3 would underflow
e4m3 otherwise); the on-chip Y state runs at 64x scale (c2 and gumbel
prescaled on host, argmax is scale-invariant) and the exact 1/64
descale rides the fp8 cast feeding each AllGather, so it costs nothing.

Strategy (8 NeuronCores, row-parallel, padded to 1280 rows/core):
 - core c owns rows [1250c, 1250c+1250), zero-padded to 1280 so the
   padded N is 10240 = 80 chunks of 128 exactly.
 - (64*B^2)^T shard (fp8 e4m3, [10240 x 1280]) stays RESIDENT in SBUF;
   all 5 passes stream it from SBUF through the tensor engine with
   Y stationary: out^T[16,1280] = Y^T @ opT, split over 4 PE column
   groups (tile_position at partitions 0/32/64/96) so the four matmuls
   of each contraction chunk stream concurrently (~3x PE throughput).
 - per-iteration AllGather of the fp8 Y slice in chunk-tiled p-major
   layout [128,10,16] so both collective-side DMAs are clean 320B-line
   transfers; big input loads are spread over the scalar/gpsimd DMA
   queues and mask prefetches are dependency-gated into pass windows so
   nothing contends with the latency-bound collectives.
 - gumbel straight-through one-hot computed LOCALLY on the core's own
   rows; the final exchange gathers the fp8 one-hot (exact in fp8).
 - final pass streams the 0/1 mask of the ORIGINAL adj (fp8): 8/10
   groups prefetched into SBUF during iteration pass windows, the last
   2 loaded into the freed operator pool slots during pass 5.
Verified on host: 1 argmax flip out of 10000 rows, final relerr ~4e-6
(tolerance 2e-2).
"""

import hashlib
import os
import shutil
import sys
from pathlib import Path

import numpy as np
import ml_dtypes

sys.path.insert(0, "/opt/trn_rl_repo")

import concourse.bass as bass  # noqa: E402
import concourse.mybir as mybir  # noqa: E402
import concourse.tile as tile  # noqa: E402
from concourse import bacc  # noqa: E402
import concourse.bass2jax as bass2jax  # noqa: E402
from concourse.bass_utils import run_bass_kernel_spmd  # noqa: E402

F8 = ml_dtypes.float8_e4m3
NCORES = 8
N = 10000
C = 16
R = N // NCORES           # 1250 real rows per core
RP = 1280                 # padded rows per core
NP = RP * NCORES          # 10240 padded N
NB = RP // 128            # 10 local blocks of 128 rows
NCH = NP // 128           # 80 contraction chunks of 128
NG = NCH // 8             # 10 adjT groups of 8 chunks
# Output halves: A = local blocks 0..5, B = blocks 6..9.  Each half is
# computed by 4 concurrent PE column-group strips (128-multiple widths
# so transpose blocks never straddle strips); the half's rows are
# AllGathered as soon as they are ready, overlapping the other half's
# matmuls and the next pass's A-dependent contraction chunks.
HA = dict(b0=0, nb=6, aw=256,
          strips=[(0, 0, 256), (32, 256, 256), (64, 512, 128), (96, 640, 128)],
          blk=[(0, 0), (0, 128), (32, 0), (32, 128), (64, 0), (96, 0)])
HB = dict(b0=6, nb=4, aw=128,
          strips=[(0, 768, 128), (32, 896, 128), (64, 1024, 128),
                  (96, 1152, 128)],
          blk=[(0, 0), (32, 0), (64, 0), (96, 0)])
# contraction chunk orders: natural for pass 0 (tracks HBM arrival),
# A-covered chunks first for later passes (they only need the A-half
# AllGather of the previous pass)
ORDER0 = list(range(NCH))
ORDER_DEP = ([10 * c + b for c in range(NCORES) for b in range(6)]
             + [10 * c + b for c in range(NCORES) for b in range(6, 10)])
MRES = 8                  # mask groups resident in SBUF
NITER = 5                 # fused 2-step passes (10 reference steps)
SCALE = 64.0              # fp8 scale for B^2 (descaled in the AG cast)
DR = False                # fp8 DoubleRow matmuls (K=256 per instruction)
SPLIT_AG = False          # per-half AllGathers (False: one full AG/pass)

_NEFF_CACHE = Path.home() / ".cache" / "bass_neff"


def _install_neff_cache():
    orig = bass2jax.compile_bir_kernel
    if getattr(bass2jax.compile_bir_kernel, "_cached", False):
        return

    def cached(bir_json, tmpdir, neff_name="file.neff"):
        h = hashlib.sha256(bir_json).hexdigest()
        p = _NEFF_CACHE / f"{h}.neff"
        dst = os.path.join(tmpdir, neff_name)
        if p.exists():
            shutil.copy(p, dst)
            return dst
        out = orig(bir_json, tmpdir, neff_name)
        try:
            _NEFF_CACHE.mkdir(parents=True, exist_ok=True)
            shutil.copy(out, p)
        except OSError:
            pass
        return out

    cached._cached = True
    bass2jax.compile_bir_kernel = cached


def build_program():
    nc = bacc.Bacc(
        "TRN2", target_bir_lowering=False, debug=False,
        enable_asserts=False, num_devices=NCORES,
    )
    f8, f16, f32 = mybir.dt.float8e4, mybir.dt.float16, mybir.dt.float32
    u8 = mybir.dt.uint8

    # pre-tiled p-major [128, chunk, col] so group loads are contiguous
    adjT_d = nc.dram_tensor("adjT8", [128, NCH * RP], f8, kind="ExternalInput")
    maskT_d = nc.dram_tensor("maskT8", [128, NCH * RP], f8, kind="ExternalInput")
    y0_d = nc.dram_tensor("y0t", [128, NCH * C], f8, kind="ExternalInput")
    guml_d = nc.dram_tensor("gumloc", [128, NB * C], f32, kind="ExternalInput")
    c2_d = nc.dram_tensor("c2loc", [128, NB * C], f16, kind="ExternalInput")
    lloc_d = nc.dram_tensor("lloc", [128, NB * C], f16, kind="ExternalInput")
    mloc_d = nc.dram_tensor("mloc", [128, NB * C], u8, kind="ExternalInput")
    pst_d = nc.dram_tensor("pst", [128, NB * C], f32, kind="ExternalInput")
    id16_d = nc.dram_tensor("id416", [128, C], f16, kind="ExternalInput")
    id32_d = nc.dram_tensor("id432", [128, C], f32, kind="ExternalInput")
    out_d = nc.dram_tensor("out_sq", [128, NB], f32, kind="ExternalOutput")
    DBG = False
    if DBG:
        dbg_y1_d = nc.dram_tensor("dbg_y1", [128, NB * C], f16,
                                  kind="ExternalOutput")
        dbg_yc_d = nc.dram_tensor("dbg_yc", [128, NCH * C], f8,
                                  kind="ExternalOutput")
        dbg_y5_d = nc.dram_tensor("dbg_y5", [128, NB * C], f16,
                                  kind="ExternalOutput")
        dbg_yh_d = nc.dram_tensor("dbg_yh", [128, NB * C], f16,
                                  kind="ExternalOutput")
        dbg_dist_d = nc.dram_tensor("dbg_dist", [128, NB * C], f32,
                                    kind="ExternalOutput")

    with tile.TileContext(nc) as tc:
        with (
            tc.tile_pool(name="sb", bufs=1) as sb,
            tc.tile_pool(name="ps", bufs=2, space="PSUM") as ps,
            tc.tile_pool(name="dram", bufs=2, space="DRAM") as dram,
        ):
            # ---- resident tiles -------------------------------------
            at_g = [sb.tile([128, 8 * RP], f8, name=f"at{g}", tag=f"at{g}")
                    for g in range(NG)]
            mt_res = [sb.tile([128, 8 * RP], f8, name=f"mt{g}", tag=f"mt{g}")
                      for g in range(MRES)]
            ycur = sb.tile([128, NCH * C], f8)
            yT_A = sb.tile([128, 256], f16)
            yT_B = sb.tile([128, 128], f16)
            dT_A = sb.tile([128, 256], f32)
            dT_B = sb.tile([128, 128], f32)
            yloc = sb.tile([128, NB * C], f16)
            yloc8 = sb.tile([128, NB * C], f8)
            gumloc = sb.tile([128, NB * C], f32)
            c2loc = sb.tile([128, NB * C], f16)
            lloc = sb.tile([128, NB * C], f16)
            mloc = sb.tile([128, NB * C], u8)
            pst = sb.tile([128, NB * C], f32)
            id16 = sb.tile([128, C], f16)
            id32 = sb.tile([128, C], f32)
            logl = sb.tile([128, NB, C], f32)
            rmax = sb.tile([128, NB], f32)
            yh16 = sb.tile([128, NB * C], f16)

            # ---- initial loads --------------------------------------
            # small tensors first (sync queue); adjT group loads on two
            # queues, even/odd interleaved so arrival tracks consumption
            nc.sync.dma_start(out=ycur[:], in_=y0_d[:])
            nc.sync.dma_start(out=id16[:], in_=id16_d[:])
            nc.sync.dma_start(out=c2loc[:], in_=c2_d[:])
            nc.sync.dma_start(out=lloc[:], in_=lloc_d[:])
            nc.sync.dma_start(out=mloc[:], in_=mloc_d[:])
            nc.sync.dma_start(out=gumloc[:], in_=guml_d[:])
            nc.sync.dma_start(out=pst[:], in_=pst_d[:])
            nc.sync.dma_start(out=id32[:], in_=id32_d[:])
            # operator loads on all 3 DMA-capable queues (each sustains
            # ~100GB/s; the HBM roofline needs all of them), interleaved
            # so arrival order tracks pass-0 consumption
            load_engs = [nc.scalar, nc.gpsimd, nc.sync]
            for g in range(NG):
                load_engs[g % 3].dma_start(
                    out=at_g[g][:],
                    in_=adjT_d[:, g * 8 * RP:(g + 1) * 8 * RP],
                )

            def mm_half(acc, lhs_tile, chunk_tile, half, order, lo, hi):
                """Issue order[lo:hi] contraction chunks for one half.

                Each chunk is 4 column-group matmuls (tile_position at
                partitions 0/32/64/96) streaming concurrently.
                """
                n = len(order)
                for idx in range(lo, hi):
                    k = order[idx]
                    g, j = divmod(k, 8)
                    lhsT = lhs_tile[:, k * C:(k + 1) * C]
                    rt = chunk_tile(g)
                    for (pb, co, w) in half["strips"]:
                        nc.tensor.matmul(
                            acc[pb:pb + C, 0:w],
                            lhsT,
                            rt[:, j * RP + co:j * RP + co + w],
                            start=(idx == 0), stop=(idx == n - 1),
                            tile_position=(0, pb),
                            # CoreSim's zero-region tracker ignores the
                            # partition offset; concurrent column strips
                            # in one bank are fine on HW
                            skip_group_check=True,
                        )

            def copies_half(dst, acc, half):
                """psum strips -> sbuf, alternating scalar/vector."""
                for i, (pb, co, w) in enumerate(half["strips"]):
                    if i % 2 == 0:
                        nc.scalar.copy(dst[pb:pb + C, 0:w],
                                       acc[pb:pb + C, 0:w])
                    else:
                        nc.vector.tensor_copy(dst[pb:pb + C, 0:w],
                                              acc[pb:pb + C, 0:w])

            def transposes_half(trp, src, ident, half, dst_b0=0):
                for bi in range(half["nb"]):
                    pb, o = half["blk"][bi]
                    nc.tensor.transpose(
                        trp[:, (dst_b0 + bi) * C:(dst_b0 + bi + 1) * C],
                        src[pb:pb + C, o:o + 128],
                        ident[pb:pb + C, :],
                        tile_position=(pb, 0),
                    )

            def ag_start(tag, half, name):
                """Launch AllGather of this half's fp8 rows (no scatter)."""
                b0, nb = half["b0"], half["nb"]
                sl = slice(b0 * C, (b0 + nb) * C)
                cc_in = dram.tile([128, nb * C], f8, name=f"ci{name}",
                                  tag=f"ccin{tag}")
                cc_out = dram.tile([NCORES * 128, nb * C], f8,
                                   name=f"co{name}", tag=f"ccout{tag}",
                                   addr_space="Shared")
                nc.sync.dma_start(out=cc_in[:], in_=yloc8[:, sl])
                nc.gpsimd.collective_compute(
                    "AllGather", mybir.AluOpType.bypass,
                    replica_groups=[list(range(NCORES))],
                    ins=[cc_in[:]], outs=[cc_out[:]],
                )
                return cc_out

            def ag_scatter(cc_out, half):
                """Scatter gathered rows into ycur.

                MUST be issued after every matmul of the current pass:
                it overwrites ycur regions the pass still reads (the
                byte-range tracker fences write-after-read).
                """
                b0, nb = half["b0"], half["nb"]
                nc.sync.dma_start(
                    out=ycur[:].rearrange("p (g x) -> p g x",
                                          g=NCORES)[:, :,
                                                    b0 * C:(b0 + nb) * C],
                    in_=cc_out[:].rearrange("(g p) x -> p g x", p=128),
                )

            def gumbel_half(half):
                """Straight-through one-hot on this half's local rows."""
                b0, nb = half["b0"], half["nb"]
                sl = slice(b0 * C, (b0 + nb) * C)
                nc.vector.tensor_tensor(
                    logl[:, b0:b0 + nb, :].rearrange("p b c -> p (b c)"),
                    yloc[:, sl], gumloc[:, sl], mybir.AluOpType.add,
                )
                nc.vector.tensor_reduce(
                    rmax[:, b0:b0 + nb], logl[:, b0:b0 + nb, :],
                    axis=mybir.AxisListType.X, op=mybir.AluOpType.max,
                )
                nc.vector.tensor_tensor(
                    yh16[:, sl].rearrange("p (b c) -> p b c", c=C),
                    logl[:, b0:b0 + nb, :],
                    rmax[:, b0:b0 + nb].unsqueeze(2).broadcast_to(
                        [128, nb, C]),
                    mybir.AluOpType.is_equal,
                )
                nc.vector.copy_predicated(yh16[:, sl], mloc[:, sl],
                                          lloc[:, sl])
                nc.vector.tensor_copy(yloc8[:, sl], yh16[:, sl])

            # ---- 5 fused 2-step propagation passes ------------------
            # Per pass: compute half A (blocks 0-5), AllGather it while
            # the PE runs half B; the next pass's contraction starts on
            # the A-covered chunks as soon as scatter-A lands.  The last
            # pass runs the gumbel one-hot per half, feeding the final
            # one-hot AllGathers the same way; the mask pass consumes
            # them in A-then-B chunk order.
            mt_s = []
            for t in range(NITER):
                last = t == NITER - 1
                order = ORDER0 if t == 0 else ORDER_DEP
                # each PSUM accumulation tile padded to a full 2KB bank:
                # two pending accumulation groups must not share a bank
                accA = ps.tile([128, 512], f32, name=f"accA{t}",
                               tag="accA", bufs=1)
                accB = ps.tile([128, 512], f32, name=f"accB{t}",
                               tag="accB", bufs=1)
                at = lambda g: at_g[g]
                slA = slice(0, HA["nb"] * C)
                slB = slice(HB["b0"] * C, NB * C)
                mm_half(accA, ycur, at, HA, order, 0, NCH)
                copies_half(yT_A, accA, HA)
                # a few B chunks before the A transposes so the PE never
                # waits on the A strip copies
                mm_half(accB, ycur, at, HB, order, 0, 12)
                trpA = ps.tile([128, 1024], f16, name=f"trpA{t}",
                               tag="trpA", bufs=1)
                transposes_half(trpA, yT_A, id16, HA)
                # 64Y <- (64 B^2) Y + 64(Bc + c)
                nc.vector.tensor_tensor(yloc[:, slA],
                                        trpA[:, 0:HA["nb"] * C],
                                        c2loc[:, slA], mybir.AluOpType.add)
                if not last:
                    # yloc carries 64*Y; descale exactly in the fp8 cast
                    nc.vector.tensor_scalar_mul(yloc8[:, slA], yloc[:, slA],
                                                1.0 / SCALE)
                else:
                    gumbel_half(HA)
                ccA = ag_start("A", HA, f"A{t}") if SPLIT_AG else None
                # ---- half B (still reads the PRE-scatter ycur) ----
                mm_half(accB, ycur, at, HB, order, 12, NCH)
                copies_half(yT_B, accB, HB)
                trpB = ps.tile([128, 1024], f16, name=f"trpB{t}",
                               tag="trpB", bufs=1)
                transposes_half(trpB, yT_B, id16, HB)
                # all of this pass's reads are issued; scatter may land
                if SPLIT_AG:
                    ag_scatter(ccA, HA)
                nc.vector.tensor_tensor(yloc[:, slB],
                                        trpB[:, 0:HB["nb"] * C],
                                        c2loc[:, slB], mybir.AluOpType.add)
                if not last:
                    nc.vector.tensor_scalar_mul(yloc8[:, slB], yloc[:, slB],
                                                1.0 / SCALE)
                else:
                    gumbel_half(HB)
                    # last two mask groups reuse operator pool slots whose
                    # final reads are in this pass
                    for qi, g in enumerate(range(MRES, NG)):
                        mt = sb.tile([128, 8 * RP], f8, name=f"mts{g}",
                                     tag=f"at{g}")
                        (nc.scalar if qi == 0 else nc.gpsimd).dma_start(
                            out=mt[:],
                            in_=maskT_d[:, g * 8 * RP:(g + 1) * 8 * RP],
                        )
                        mt_s.append(mt)
                if SPLIT_AG:
                    ccB = ag_start("B", HB, f"B{t}")
                    ag_scatter(ccB, HB)
                else:
                    cc_in = dram.tile([128, NB * C], f8, name=f"cif{t}",
                                      tag="ccinA")
                    cc_out = dram.tile([NCORES * 128, NB * C], f8,
                                       name=f"cof{t}", tag="ccoutA",
                                       addr_space="Shared")
                    nc.sync.dma_start(out=cc_in[:], in_=yloc8[:])
                    nc.gpsimd.collective_compute(
                        "AllGather", mybir.AluOpType.bypass,
                        replica_groups=[list(range(NCORES))],
                        ins=[cc_in[:]], outs=[cc_out[:]],
                    )
                    nc.sync.dma_start(
                        out=ycur[:].rearrange("p (g x) -> p g x", g=NCORES),
                        in_=cc_out[:].rearrange("(g p) x -> p g x", p=128),
                    )
                if not last:
                    # mask prefetch, gated on the post-collective gather
                    # (tiny write into the target tile) so the load runs
                    # during the NEXT pass, when HBM is otherwise idle.
                    # Issued last so nothing queues behind the gates.
                    for qi, mg in enumerate((2 * t, 2 * t + 1)):
                        mt = mt_res[mg]
                        nc.vector.tensor_copy(mt[0:1, 0:4], ycur[0:1, 0:4])
                        (nc.scalar if qi == 0 else nc.gpsimd).dma_start(
                            out=mt[:],
                            in_=maskT_d[:, mg * 8 * RP:(mg + 1) * 8 * RP],
                        )
                if DBG and t == 0:
                    nc.sync.dma_start(out=dbg_y1_d[:], in_=yloc[:])
                    nc.sync.dma_start(out=dbg_yc_d[:], in_=ycur[:])
                if DBG and last:
                    nc.sync.dma_start(out=dbg_y5_d[:], in_=yloc[:])
                    nc.sync.dma_start(out=dbg_yh_d[:], in_=yh16[:])

            # ---- final pass: dist^T = Yh^T @ maskT ------------------
            mk = lambda g: mt_res[g] if g < MRES else mt_s[g - MRES]
            daccA = ps.tile([128, 512], f32, tag="accA", bufs=1)
            daccB = ps.tile([128, 512], f32, tag="accB", bufs=1)
            mm_half(daccA, ycur, mk, HA, ORDER_DEP, 0, NCH)
            copies_half(dT_A, daccA, HA)
            mm_half(daccB, ycur, mk, HB, ORDER_DEP, 0, NCH)
            copies_half(dT_B, daccB, HB)

            # ---- normalize + squared error --------------------------
            trd = ps.tile([128, 512], f32, tag="trd", bufs=1)
            transposes_half(trd, dT_A, id32, HA, dst_b0=0)
            transposes_half(trd, dT_B, id32, HB, dst_b0=HB["b0"])
            dist = sb.tile([128, NB, C], f32)
            nc.vector.tensor_copy(dist[:].rearrange("p b c -> p (b c)"),
                                  trd[:, 0:NB * C])
            rsum = sb.tile([128, NB], f32)
            nc.vector.tensor_reduce(
                rsum[:], dist[:], axis=mybir.AxisListType.X,
                op=mybir.AluOpType.add,
            )
            # valid rows always have rsum >= 1 (self-loop); clamp the
            # zero pad rows so 1/rsum stays finite (their dist is 0)
            nc.vector.tensor_scalar_max(rsum[:], rsum[:], 0.5)
            rinv = sb.tile([128, NB], f32)
            nc.vector.reciprocal(rinv[:], rsum[:])
            dd = sb.tile([128, NB, C], f32)
            nc.vector.tensor_tensor(
                dd[:], dist[:],
                rinv[:].unsqueeze(2).broadcast_to([128, NB, C]),
                mybir.AluOpType.mult,
            )
            if DBG:
                nc.sync.dma_start(out=dbg_dist_d[:],
                                  in_=dd[:].rearrange("p b c -> p (b c)"))
            nc.vector.tensor_tensor(
                dd[:].rearrange("p b c -> p (b c)"),
                dd[:].rearrange("p b c -> p (b c)"), pst[:],
                mybir.AluOpType.subtract,
            )
            nc.vector.tensor_tensor(
                dd[:], dd[:], dd[:], mybir.AluOpType.mult,
            )
            osq = sb.tile([128, NB], f32)
            nc.vector.tensor_reduce(
                osq[:], dd[:], axis=mybir.AxisListType.X,
                op=mybir.AluOpType.add,
            )
            nc.sync.dma_start(out=out_d[:], in_=osq[:])

    nc.compile()
    return nc


_nc = None


def _get_program():
    global _nc
    if _nc is None:
        _install_neff_cache()
        _nc = build_program()
    return _nc


def _tile_local(x, dtype):
    """[1250, cols] local slice -> [128, 10*cols] chunk-tiled, padded."""
    cols = x.shape[1]
    p = np.zeros((RP, cols), np.float32)
    p[:R] = x
    return np.ascontiguousarray(
        p.reshape(NB, 128, cols).transpose(1, 0, 2).reshape(128, NB * cols)
    ).astype(dtype)


def prep_inputs(adj, labels_onehot, pseudo_labels, gumbel, train_mask):
    import scipy.sparse as sp

    adj = np.asarray(adj, np.float32)
    labels = np.asarray(labels_onehot, np.float32)
    pseudo = np.asarray(pseudo_labels, np.float32)
    gumbel = np.asarray(gumbel, np.float32)
    m = np.asarray(train_mask).astype(bool)

    labm = labels * m[:, None]
    # fused 2-step operator: B = diag(1-m) adj (sparse ~30 nnz/row),
    # B2 = B @ B (~810 nnz/row), c2 = B c + c
    Bs = sp.csr_matrix(adj)
    keep = np.repeat(~m, np.diff(Bs.indptr))
    Bs.data = np.where(keep, Bs.data, 0.0).astype(np.float32)
    B2 = np.asarray((Bs @ Bs).todense(), np.float32)
    B2 *= SCALE
    c2 = np.asarray(Bs @ labm + labm, np.float32)

    # initial Y in full padded chunk-tiled layout [128, 80*16]
    y0p = np.zeros((NP, C), np.float32)
    y0p.reshape(NCORES, RP, C)[:, :R] = labm.reshape(NCORES, R, C)
    y0t = np.ascontiguousarray(
        y0p.reshape(NCH, 128, C).transpose(1, 0, 2).reshape(128, NCH * C)
    ).astype(F8)

    # transpose identities (the HW transpose path ignores non-1 values,
    # so the SCALE descale lives in the fp8 cast / prescaled inputs)
    id16 = np.zeros((128, C), np.float16)
    id32 = np.zeros((128, C), np.float32)
    for s in range(4):
        for i in range(C):
            id16[32 * s + i, i] = 1.0
            id32[32 * s + i, i] = 1.0

    def tileT(M):
        """[R, N] row-block -> padded p-major [128, NCH*RP]."""
        blk = np.ascontiguousarray(M.T)                     # [N, R]
        padT = np.zeros((NCORES, RP, RP), np.float32)
        padT[:, :R, :R] = blk.reshape(NCORES, R, R)
        return np.ascontiguousarray(
            padT.reshape(NCH, 128, RP).transpose(1, 0, 2).reshape(128, NCH * RP)
        )

    in_maps = []
    for c in range(NCORES):
        rows = slice(c * R, (c + 1) * R)
        adjT8 = tileT(B2[rows, :]).astype(F8)
        maskT8 = (tileT(adj[rows, :]) != 0).astype(F8)
        # on-chip Y state is carried at 64x scale (argmax of the gumbel
        # logits is scale-invariant, so gumbel is prescaled to match)
        gl = _tile_local(gumbel[rows] * SCALE, np.float32)
        c2l = _tile_local(c2[rows] * SCALE, np.float16)
        ll = _tile_local(labm[rows], np.float16)
        ml = _tile_local(np.repeat(m[rows, None].astype(np.float32), C, 1),
                         np.uint8)
        pl = _tile_local(pseudo[rows], np.float32)
        in_maps.append({
            "adjT8": adjT8, "maskT8": maskT8, "y0t": y0t, "gumloc": gl,
            "c2loc": c2l, "lloc": ll, "mloc": ml, "pst": pl,
            "id416": id16, "id432": id32,
        })
    return in_maps


def run_on_device(in_maps, trace=False, **kw):
    nc = _get_program()
    return run_bass_kernel_spmd(nc, in_maps, list(range(NCORES)), trace=trace, **kw)


def kernel(adj, labels_onehot, pseudo_labels, gumbel, train_mask,
           iter_step=10, k_hop=1, **_unused):
    assert int(iter_step) == 10 and int(k_hop) == 1, "kernel hardcodes 10/1"
    in_maps = prep_inputs(adj, labels_onehot, pseudo_labels, gumbel, train_mask)
    res = run_on_device(in_maps)
    total = 0.0
    for c in range(NCORES):
        sq = np.asarray(res.results[c]["out_sq"], np.float64)
        total += sq.sum()
    return np.float32(total / (N * C))

